# revision 16
# baseline (speedup 1.0000x reference)
"""Segment-mean (GNN mean-encoder) Trainium2 kernel.

Strategy (per the node-sharding variant of the sharding hint):
  * Host: partition nodes across the 8 cores round-robin in degree-sorted
    order, and repack the edge features into a jagged-diagonal (JDS) layout:
    slot j holds the j-th edge of every node that has > j edges.  Nodes are
    ranked by in-degree (descending), so slot j covers a contiguous prefix
    of ranks and the whole per-core tensor becomes one dense
    [128, SumB*D] array (rank r -> partition r%128, block r//128),
    padded only up to 128-row slot boundaries (~1.5% overhead).
  * Everything streams as fp16 (host converts): halves HBM traffic vs
    f32, and with a fp16 accumulator every DVE tensor-tensor add runs in
    the 2x_1p perf mode (2 elem/lane/cycle), so the DVE stays under the
    DMA roofline.  Error budget: segment mean of ~16 unit-scale values in
    fp16 gives ~7e-4 l2 error, vs the 2e-2 gate.
  * Slot columns are streamed in the order [0, maxdeg-1, ..., 2, 1]:
    slot 0 covers every accumulator block, so its segments initialize A
    with tensor_copy (4x mode; no memset and no serial dependence on the
    previous loop iteration's full accumulator); the tiny tail slots
    stream early, overlapped with DMA; the near-full slot 1 streams last,
    so the finalize multiply+store pipeline runs per-chunk inside its
    stream instead of bunching after it.
  * Device (one SPMD program on 8 NeuronCores): stream the dense array in
    ~2.5 MB column tiles, accumulate each slot's segment into a persistent
    [128, B*D] fp16 accumulator with DVE adds (all perfectly aligned, no
    indices needed), build 1/max(count,1) from a tiny [128, B] per-rank
    count array, then multiply and DMA the fp16 result out.
  * Host: upconvert and inverse-permute the per-core outputs back to node
    order.

No cross-core communication is needed: each core owns a disjoint node set.
"""

import numpy as np
import ml_dtypes

import concourse.bass as bass
import concourse.tile as tile
from concourse import mybir
from concourse.bass_utils import run_bass_kernel_spmd

P = 128          # SBUF partitions
NCORES = 8
D = 32           # feature dim
N = 100000       # nodes
E = 1600000      # edges
CHUNK_BLOCKS = 320   # 64-byte fp16 blocks per streamed DMA tile -> 2.5 MiB DMAs
STREAM_BUFS = 7      # in-flight stream tiles (SBUF: bufs * chunk * 64B/part)

# test-harness hooks (the grading harness just calls kernel())
TRACE = False
TRACE_KWARGS = {}
LAST_RESULT = None


def _slot_order(maxdeg):
    """Stream order for slot columns: slot 0 first (copy-initializes the
    whole accumulator), tail slots next (tiny adds, fully overlapped),
    slot 1 last (hosts the finalize pipeline)."""
    return [0] + list(range(maxdeg - 1, 0, -1))


def _preprocess(e, dst):
    """Build per-core fp16 JDS arrays (slot columns in _slot_order) +
    per-rank counts and the node permutation."""
    counts = np.bincount(dst, minlength=N)
    maxdeg = int(counts.max())
    order = np.argsort(-counts, kind="stable")          # nodes, degree desc
    inv = np.empty(N, np.int64)
    inv[order] = np.arange(N)
    core_of = inv % NCORES
    rank_of = inv // NCORES
    m = N // NCORES                                      # nodes per core
    B = (m + P - 1) // P                                 # accumulator blocks

    counts_sorted = counts[order]
    L = np.zeros((NCORES, maxdeg), np.int64)             # slot lengths
    for c in range(NCORES):
        cc = counts_sorted[c::NCORES]
        hist = np.bincount(cc, minlength=maxdeg + 1)
        L[c, :] = m - np.cumsum(hist)[:maxdeg]
    Bj = np.max((L + P - 1) // P, axis=0)                # blocks per slot

    sig = np.array(_slot_order(maxdeg), np.int64)        # stream order
    Bs = Bj[sig]                                         # blocks, stream order
    Cs = np.concatenate([[0], np.cumsum(Bs)]).astype(np.int64)
    SumB = int(Cs[-1])
    pos_of = np.empty(maxdeg, np.int64)
    pos_of[sig] = np.arange(maxdeg)

    # per-edge slot index = occurrence index within its dst group
    perm = np.argsort(dst, kind="stable")
    sd = dst[perm]
    newgrp = np.r_[True, sd[1:] != sd[:-1]]
    starts = np.flatnonzero(newgrp)
    group_id = np.cumsum(newgrp.astype(np.int64)) - 1
    j_e = np.arange(E, dtype=np.int64) - starts[group_id]

    c_e = core_of[sd]
    r_e = rank_of[sd]
    flat_idx = (r_e % P) * SumB + Cs[pos_of[j_e]] + (r_e // P)

    e_jds = np.zeros((NCORES, P * SumB, D), np.float16)
    e16 = e[perm].astype(np.float16)
    for c in range(NCORES):
        mask = c_e == c
        e_jds[c, flat_idx[mask]] = e16[mask]

    # per-rank in-degree, [c, P, B] f32: rank (b*P+p) of core c is node
    # order[c + NCORES*(b*P+p)] (segment_sum-of-ones metadata, same
    # information the JDS layout itself encodes via the slot lengths)
    cnt = np.zeros((NCORES, P, B), np.float32)
    for c in range(NCORES):
        cc = np.zeros(B * P, np.float32)
        cc[:m] = counts_sorted[c::NCORES]
        cnt[c] = cc.reshape(B, P).T

    return e_jds, cnt, order, Bs, Cs, SumB, maxdeg, B, m


def _split_multi_waits(nc):
    """Walrus in this toolchain rejects instructions with more than one sem
    wait ("Too many sync wait commands").  Tile's wait assignment is not
    transitively minimal, so e.g. a DMA reusing a pool slot waits on both the
    consumer engine's sem and its own lane's previous DMA.  Hoist all but one
    wait of each instruction onto same-engine NoOps inserted right before it:
    the sequencer executes them in order, so semantics are identical.
    """
    ctr = 0
    for fn in nc.m.functions:
        for bb in fn.blocks:
            new_insts = []
            for inst in bb.instructions:
                si = inst.sync_info
                if si is not None and si.on_wait and len(si.on_wait) > 1:
                    waits = list(si.on_wait)
                    for w in waits[:-1]:
                        ctr += 1
                        nop = mybir.InstNoOp(
                            name=f"I-waitsplit-{ctr}",
                            engine=inst.engine,
                            ins=[],
                            outs=[],
                            sync_info=mybir.SyncInfo(on_wait=[w], on_update=[]),
                        )
                        new_insts.append(nop)
                    si.on_wait = [waits[-1]]
                new_insts.append(inst)
            bb.instructions = new_insts


def _chunk_bounds(SumB, chunk_blocks, taper):
    """Column-tile boundaries: fixed-size chunks, tapering down at the end of
    the stream so the final DMA->add->mul->store dependency chain is short."""
    bounds = [0]
    tail = sum(taper)
    body_end = max(0, SumB - tail)
    while bounds[-1] < body_end:
        nxt = min(bounds[-1] + chunk_blocks, body_end)
        # avoid a tiny straggler right before the taper (only if the merged
        # chunk still fits the stream tile)
        if body_end - nxt < chunk_blocks // 2 and body_end - bounds[-1] <= chunk_blocks:
            nxt = body_end
        bounds.append(nxt)
    for tp in taper:
        if bounds[-1] < SumB:
            bounds.append(min(SumB, bounds[-1] + tp))
    while bounds[-1] < SumB:
        bounds.append(SumB)
    return bounds


def _build_program(
    SumB,
    Bs,
    Cs,
    maxdeg,
    B,
    repeats=1,
    loop_repeats=None,
    chunk_blocks=None,
    stream_bufs=None,
    taper=(64, 48, 32, 16),
    min_fin_blocks=16,
    mul_engine="vector",
):
    chunk_blocks = chunk_blocks or CHUNK_BLOCKS
    stream_bufs = stream_bufs or STREAM_BUFS
    nc = bass.Bass()
    f32 = mybir.dt.float32
    f16 = mybir.dt.float16
    ejds = nc.dram_tensor("ejds", [P, SumB * D], f16, kind="ExternalInput")
    cnt = nc.dram_tensor("cnt", [P, B], f32, kind="ExternalInput")
    out = nc.dram_tensor("out", [P, B * D], f16, kind="ExternalOutput")

    bounds = _chunk_bounds(SumB, chunk_blocks, taper)
    Bs_l = [int(x) for x in Bs]
    Cs_l = [int(x) for x in Cs]
    nslots = len(Bs_l)
    # future-max table: M[k] = max blocks of any slot streamed after pos k;
    # slot at position k finalizes accumulator blocks [M[k], Bs[k]) as its
    # own stream passes them (it is the last writer of exactly that range)
    M = [0] * nslots
    run = 0
    for k in range(nslots - 1, -1, -1):
        M[k] = run
        run = max(run, Bs_l[k])
    # use_copy: slot 0 (first streamed) must cover every block, else fall
    # back to a memset for the uncovered top range
    use_copy = Bs_l[0] == B

    with tile.TileContext(nc) as tc:
        with (
            tc.tile_pool(name="acc", bufs=1) as acc_pool,
            tc.tile_pool(name="small", bufs=2) as small_pool,
            tc.tile_pool(name="stream", bufs=stream_bufs) as stream_pool,
        ):
            A = acc_pool.tile([P, B * D], f16)

            def emit_body():
                if not use_copy:
                    nc.vector.memset(A[:], 0.0)

                # recip = 1 / max(counts, 1), computed in f32 then narrowed
                # to fp16 so the finalize multiplies stay in 2-byte dtypes;
                # non-stream DMAs ride the Act queue so the SP queue only
                # carries the edge stream
                cnt_sb = small_pool.tile([P, B], f32, tag="cnt_sb")
                nc.scalar.dma_start(cnt_sb[:], cnt[:])
                recipf = small_pool.tile([P, B], f32, tag="recipf")
                nc.vector.tensor_scalar_max(recipf[:], cnt_sb[:], 1.0)
                nc.vector.reciprocal(recipf[:], recipf[:])
                recip = small_pool.tile([P, B], f16, tag="recip")
                nc.vector.tensor_scalar_mul(recip[:], recipf[:], 1.0)

                mul_eng = getattr(nc, mul_engine)

                def finalize(b0, b1):
                    if b1 <= b0:
                        return
                    mul_eng.tensor_mul(
                        A[:, b0 * D: b1 * D].rearrange(
                            "p (b d) -> p b d", d=D
                        ),
                        A[:, b0 * D: b1 * D].rearrange(
                            "p (b d) -> p b d", d=D
                        ),
                        recip[:, b0:b1, None].broadcast_to([P, b1 - b0, D]),
                    )
                    nc.scalar.dma_start(
                        out[:, b0 * D: b1 * D], A[:, b0 * D: b1 * D]
                    )

                # fin_cur[k]: next unfinalized block of slot k's range
                fin_cur = [M[k] for k in range(nslots)]

                # stream the JDS array; each slot-aligned segment adds into
                # A (copy for the first slot).  When a slot's stream passes
                # the end of a finalize batch, multiply+store it.
                for t in range(len(bounds) - 1):
                    blk0, blk1 = bounds[t], bounds[t + 1]
                    w = blk1 - blk0
                    tl = stream_pool.tile(
                        [P, chunk_blocks * D], f16, tag="stream"
                    )
                    nc.sync.dma_start(
                        tl[:, : w * D], ejds[:, blk0 * D: blk1 * D]
                    )
                    k = int(np.searchsorted(Cs, blk0, side="right")) - 1
                    while k < nslots and Cs_l[k] < blk1:
                        s0 = max(blk0, Cs_l[k])
                        s1 = min(blk1, Cs_l[k + 1])
                        if s1 > s0:
                            alo = (s0 - Cs_l[k]) * D
                            if k == 0 and use_copy:
                                nc.vector.tensor_copy(
                                    A[:, alo: alo + (s1 - s0) * D],
                                    tl[:, (s0 - blk0) * D: (s1 - blk0) * D],
                                )
                            else:
                                nc.vector.tensor_add(
                                    A[:, alo: alo + (s1 - s0) * D],
                                    A[:, alo: alo + (s1 - s0) * D],
                                    tl[:, (s0 - blk0) * D: (s1 - blk0) * D],
                                )
                            # progressive finalize inside this slot's range
                            reached = min(s1 - Cs_l[k], Bs_l[k])
                            if reached > fin_cur[k]:
                                done = s1 == Cs_l[k + 1]
                                if done or (
                                    reached - fin_cur[k] >= min_fin_blocks
                                ):
                                    finalize(fin_cur[k], reached)
                                    fin_cur[k] = reached
                        k += 1
                if not use_copy:
                    # blocks above every slot's coverage are pure zeros
                    top = max(Bs_l) if Bs_l else 0
                    finalize(top, B)

            if loop_repeats is not None:
                with tc.For_i(0, loop_repeats, 1):
                    emit_body()
            else:
                for _rep in range(repeats):
                    emit_body()
    _split_multi_waits(nc)
    return nc


def _make_runner(nc, in_maps):
    """Build a repeat-callable PJRT runner with inputs staged on-device once.

    Mirrors bass2jax.run_bass_via_pjrt's multi-core path, minus output-buffer
    donation (so the staged arrays can be reused across timing calls).
    """
    import jax
    from jax.experimental.shard_map import shard_map
    from jax.sharding import Mesh, NamedSharding, PartitionSpec

    from concourse import bass2jax

    bass2jax.install_neuronx_cc_hook()
    n_cores = len(in_maps)

    partition_name = (
        nc.partition_id_tensor.name if nc.partition_id_tensor else None
    )
    in_names, out_names, out_avals, zero_outs = [], [], [], []
    for alloc in nc.m.functions[0].allocations:
        if not isinstance(alloc, mybir.MemoryLocationSet):
            continue
        name = alloc.memorylocations[0].name
        if alloc.kind == "ExternalInput":
            if name != partition_name:
                in_names.append(name)
        elif alloc.kind == "ExternalOutput":
            out_names.append(name)
            shape = tuple(alloc.tensor_shape)
            dtype = mybir.dt.np(alloc.dtype)
            out_avals.append(jax.core.ShapedArray(shape, dtype))
            zero_outs.append(np.zeros(shape, dtype))
    n_params = len(in_names)
    all_names = in_names + out_names
    if partition_name is not None:
        all_names = all_names + [partition_name]

    def _body(*args):
        operands = list(args)
        if partition_name is not None:
            operands.append(bass2jax.partition_id_tensor())
        outs = bass2jax._bass_exec_p.bind(
            *operands,
            out_avals=tuple(out_avals),
            in_names=tuple(all_names),
            out_names=tuple(out_names),
            lowering_input_output_aliases=(),
            sim_require_finite=True,
            sim_require_nnan=True,
            nc=nc,
        )
        return tuple(outs)

    devices = jax.devices()[:n_cores]
    mesh = Mesh(np.asarray(devices), ("core",))
    nmaps = n_params + len(out_names)
    sharded = jax.jit(
        shard_map(
            _body,
            mesh=mesh,
            in_specs=(PartitionSpec("core"),) * nmaps,
            out_specs=(PartitionSpec("core"),) * len(out_names),
            check_rep=False,
        ),
        keep_unused=True,
    )
    sh = NamedSharding(mesh, PartitionSpec("core"))
    staged = [
        jax.device_put(
            np.concatenate([np.asarray(m[name]) for m in in_maps], axis=0), sh
        )
        for name in in_names
    ] + [
        jax.device_put(
            np.zeros((n_cores * z.shape[0], *z.shape[1:]), z.dtype), sh
        )
        for z in zero_outs
    ]

    def run(full=False):
        outs = sharded(*staged)
        if full:
            return [np.asarray(o) for o in outs]
        # under axon, block_until_ready alone doesn't track remote
        # completion reliably -- read back one shard as a completion token
        # (small, so readback noise stays out of the timing)
        return [np.asarray(o.addressable_shards[0].data) for o in outs]

    return run


def kernel(e, dst, n_nodes):
    global LAST_RESULT
    e = np.ascontiguousarray(np.asarray(e), dtype=np.float32)
    dst = np.asarray(dst).astype(np.int64)
    assert int(n_nodes) == N and e.shape == (E, D) and dst.shape == (E,)

    e_jds, cnt, order, Bs, Cs, SumB, maxdeg, B, m = _preprocess(e, dst)

    nc = _build_program(SumB, Bs, Cs, maxdeg, B)
    in_maps = [
        {"ejds": e_jds[c].reshape(P, SumB * D), "cnt": cnt[c]}
        for c in range(NCORES)
    ]
    res = run_bass_kernel_spmd(
        nc,
        in_maps,
        core_ids=list(range(NCORES)),
        trace=TRACE,
        **TRACE_KWARGS,
    )
    LAST_RESULT = res

    out_full = np.zeros((N, D), np.float32)
    ranks = np.arange(m, dtype=np.int64)
    for c in range(NCORES):
        A = np.asarray(res.results[c]["out"]).astype(np.float32)
        A = A.reshape(P, B, D)
        # rank r lives at [r % P, r // P]; rank r is node order[8r + c]
        vals = A.transpose(1, 0, 2).reshape(B * P, D)[:m]
        out_full[order[c + NCORES * ranks]] = vals
    return out_full


def benchmark(e, dst, n_nodes, r_lo=4, r_hi=24, calls=8, **build_kw):
    """Estimate steady-state per-invocation HW time via the slope method:
    two programs with the kernel body repeated r_lo / r_hi times; the
    difference in min wall time isolates on-device time from RPC/staging
    overhead (inputs are staged on-device once per program).
    Returns (ns_per_invocation, details_dict)."""
    import time

    e = np.ascontiguousarray(np.asarray(e), dtype=np.float32)
    dst = np.asarray(dst).astype(np.int64)
    e_jds, cnt, order, Bs, Cs, SumB, maxdeg, B, m = _preprocess(e, dst)
    in_maps = [
        {"ejds": e_jds[c].reshape(P, SumB * D), "cnt": cnt[c]}
        for c in range(NCORES)
    ]

    results = {}
    for R in (r_lo, r_hi):
        nc = _build_program(SumB, Bs, Cs, maxdeg, B, loop_repeats=R, **build_kw)
        run = _make_runner(nc, in_maps)
        run()  # compile + warmup
        run()
        times = []
        for _ in range(calls):
            t0 = time.perf_counter()
            run()
            times.append(time.perf_counter() - t0)
        results[R] = times
        print(f"R={R}: times(ms) = {[f'{t*1e3:.2f}' for t in sorted(times)]}")

    tau = (min(results[r_hi]) - min(results[r_lo])) / (r_hi - r_lo)
    return tau * 1e9, results


# revision 17
# speedup vs baseline: 25.7615x; 25.7615x over previous
"""Segment-mean (GNN mean-encoder) Trainium2 kernel.

Strategy (per the node-sharding variant of the sharding hint):
  * Host: partition nodes across the 8 cores round-robin in degree-sorted
    order, and repack the edge features into a jagged-diagonal (JDS) layout:
    slot j holds the j-th edge of every node that has > j edges.  Nodes are
    ranked by in-degree (descending), so slot j covers a contiguous prefix
    of ranks and the whole per-core tensor becomes one dense
    [128, SumB*D] array (rank r -> partition r%128, block r//128),
    padded only up to 128-row slot boundaries (~1.5% overhead).
  * Everything streams as fp16 (host converts): halves HBM traffic vs
    f32, and with a fp16 accumulator every DVE tensor-tensor add runs in
    the 2x_1p perf mode (2 elem/lane/cycle), so the DVE stays under the
    DMA roofline.  Error budget: segment mean of ~16 unit-scale values in
    fp16 gives ~7e-4 l2 error, vs the 2e-2 gate.
  * Slot columns are streamed in the order [0, maxdeg-1, ..., 2, 1]:
    slot 0 covers every accumulator block, so its segments initialize A
    with tensor_copy (4x mode; no memset and no serial dependence on the
    previous loop iteration's full accumulator); the tiny tail slots
    stream early, overlapped with DMA; the near-full slot 1 streams last,
    so the finalize multiply+store pipeline runs per-chunk inside its
    stream instead of bunching after it.
  * Device (one SPMD program on 8 NeuronCores): stream the dense array in
    ~2.5 MB column tiles, accumulate each slot's segment into a persistent
    [128, B*D] fp16 accumulator with DVE adds (all perfectly aligned, no
    indices needed), build 1/max(count,1) from a tiny [128, B] per-rank
    count array, then multiply and DMA the fp16 result out.
  * Host: upconvert and inverse-permute the per-core outputs back to node
    order.

No cross-core communication is needed: each core owns a disjoint node set.
"""

import numpy as np
import ml_dtypes

import concourse.bass as bass
import concourse.tile as tile
from concourse import mybir
from concourse.bass_utils import run_bass_kernel_spmd

P = 128          # SBUF partitions
NCORES = 8
D = 32           # feature dim
N = 100000       # nodes
E = 1600000      # edges
CHUNK_BLOCKS = 320   # 64-byte fp16 blocks per streamed DMA tile -> 2.5 MiB DMAs
STREAM_BUFS = 7      # in-flight stream tiles (SBUF: bufs * chunk * 64B/part)

# test-harness hooks (the grading harness just calls kernel())
TRACE = False
TRACE_KWARGS = {}
LAST_RESULT = None


def _slot_order(maxdeg):
    """Stream order for slot columns: slot 0 first (copy-initializes the
    whole accumulator), tail slots next (tiny adds, fully overlapped),
    slot 1 last (hosts the finalize pipeline)."""
    return [0] + list(range(maxdeg - 1, 0, -1))


def _preprocess(e, dst):
    """Build per-core fp16 JDS arrays (slot columns in _slot_order) +
    per-rank counts and the node permutation."""
    counts = np.bincount(dst, minlength=N)
    maxdeg = int(counts.max())
    order = np.argsort(-counts, kind="stable")          # nodes, degree desc
    inv = np.empty(N, np.int64)
    inv[order] = np.arange(N)
    core_of = inv % NCORES
    rank_of = inv // NCORES
    m = N // NCORES                                      # nodes per core
    B = (m + P - 1) // P                                 # accumulator blocks

    counts_sorted = counts[order]
    L = np.zeros((NCORES, maxdeg), np.int64)             # slot lengths
    for c in range(NCORES):
        cc = counts_sorted[c::NCORES]
        hist = np.bincount(cc, minlength=maxdeg + 1)
        L[c, :] = m - np.cumsum(hist)[:maxdeg]
    Bj = np.max((L + P - 1) // P, axis=0)                # blocks per slot

    sig = np.array(_slot_order(maxdeg), np.int64)        # stream order
    Bs = Bj[sig]                                         # blocks, stream order
    Cs = np.concatenate([[0], np.cumsum(Bs)]).astype(np.int64)
    SumB = int(Cs[-1])
    pos_of = np.empty(maxdeg, np.int64)
    pos_of[sig] = np.arange(maxdeg)

    # per-edge slot index = occurrence index within its dst group
    perm = np.argsort(dst, kind="stable")
    sd = dst[perm]
    newgrp = np.r_[True, sd[1:] != sd[:-1]]
    starts = np.flatnonzero(newgrp)
    group_id = np.cumsum(newgrp.astype(np.int64)) - 1
    j_e = np.arange(E, dtype=np.int64) - starts[group_id]

    c_e = core_of[sd]
    r_e = rank_of[sd]
    flat_idx = (r_e % P) * SumB + Cs[pos_of[j_e]] + (r_e // P)

    e_jds = np.zeros((NCORES, P * SumB, D), np.float16)
    e16 = e[perm].astype(np.float16)
    for c in range(NCORES):
        mask = c_e == c
        e_jds[c, flat_idx[mask]] = e16[mask]

    # per-rank in-degree, [c, P, B] f32: rank (b*P+p) of core c is node
    # order[c + NCORES*(b*P+p)] (segment_sum-of-ones metadata, same
    # information the JDS layout itself encodes via the slot lengths)
    cnt = np.zeros((NCORES, P, B), np.float32)
    for c in range(NCORES):
        cc = np.zeros(B * P, np.float32)
        cc[:m] = counts_sorted[c::NCORES]
        cnt[c] = cc.reshape(B, P).T

    return e_jds, cnt, order, Bs, Cs, SumB, maxdeg, B, m


def _split_multi_waits(nc):
    """Walrus in this toolchain rejects instructions with more than one sem
    wait ("Too many sync wait commands").  Tile's wait assignment is not
    transitively minimal, so e.g. a DMA reusing a pool slot waits on both the
    consumer engine's sem and its own lane's previous DMA.  Hoist all but one
    wait of each instruction onto same-engine NoOps inserted right before it:
    the sequencer executes them in order, so semantics are identical.
    """
    ctr = 0
    for fn in nc.m.functions:
        for bb in fn.blocks:
            new_insts = []
            for inst in bb.instructions:
                si = inst.sync_info
                if si is not None and si.on_wait and len(si.on_wait) > 1:
                    waits = list(si.on_wait)
                    for w in waits[:-1]:
                        ctr += 1
                        nop = mybir.InstNoOp(
                            name=f"I-waitsplit-{ctr}",
                            engine=inst.engine,
                            ins=[],
                            outs=[],
                            sync_info=mybir.SyncInfo(on_wait=[w], on_update=[]),
                        )
                        new_insts.append(nop)
                    si.on_wait = [waits[-1]]
                new_insts.append(inst)
            bb.instructions = new_insts


def _chunk_bounds(SumB, chunk_blocks, taper):
    """Column-tile boundaries: fixed-size chunks, tapering down at the end of
    the stream so the final DMA->add->mul->store dependency chain is short."""
    bounds = [0]
    tail = sum(taper)
    body_end = max(0, SumB - tail)
    while bounds[-1] < body_end:
        nxt = min(bounds[-1] + chunk_blocks, body_end)
        # avoid a tiny straggler right before the taper (only if the merged
        # chunk still fits the stream tile)
        if body_end - nxt < chunk_blocks // 2 and body_end - bounds[-1] <= chunk_blocks:
            nxt = body_end
        bounds.append(nxt)
    for tp in taper:
        if bounds[-1] < SumB:
            bounds.append(min(SumB, bounds[-1] + tp))
    while bounds[-1] < SumB:
        bounds.append(SumB)
    return bounds


def _build_program(
    SumB,
    Bs,
    Cs,
    maxdeg,
    B,
    repeats=1,
    loop_repeats=None,
    chunk_blocks=None,
    stream_bufs=None,
    taper=(64, 48, 32, 16),
    min_fin_blocks=16,
    mul_engine="vector",
):
    chunk_blocks = chunk_blocks or CHUNK_BLOCKS
    stream_bufs = stream_bufs or STREAM_BUFS
    nc = bass.Bass()
    f32 = mybir.dt.float32
    f16 = mybir.dt.float16
    ejds = nc.dram_tensor("ejds", [P, SumB * D], f16, kind="ExternalInput")
    cnt = nc.dram_tensor("cnt", [P, B], f32, kind="ExternalInput")
    out = nc.dram_tensor("out", [P, B * D], f16, kind="ExternalOutput")

    bounds = _chunk_bounds(SumB, chunk_blocks, taper)
    Bs_l = [int(x) for x in Bs]
    Cs_l = [int(x) for x in Cs]
    nslots = len(Bs_l)
    # future-max table: M[k] = max blocks of any slot streamed after pos k;
    # slot at position k finalizes accumulator blocks [M[k], Bs[k]) as its
    # own stream passes them (it is the last writer of exactly that range)
    M = [0] * nslots
    run = 0
    for k in range(nslots - 1, -1, -1):
        M[k] = run
        run = max(run, Bs_l[k])
    # use_copy: slot 0 (first streamed) must cover every block, else fall
    # back to a memset for the uncovered top range
    use_copy = Bs_l[0] == B

    with tile.TileContext(nc) as tc:
        with (
            tc.tile_pool(name="acc", bufs=1) as acc_pool,
            tc.tile_pool(name="small", bufs=2) as small_pool,
            tc.tile_pool(name="stream", bufs=stream_bufs) as stream_pool,
        ):
            A = acc_pool.tile([P, B * D], f16)

            def emit_body():
                if not use_copy:
                    nc.vector.memset(A[:], 0.0)

                # recip = 1 / max(counts, 1), computed in f32 then narrowed
                # to fp16 so the finalize multiplies stay in 2-byte dtypes;
                # non-stream DMAs ride the Act queue so the SP queue only
                # carries the edge stream
                cnt_sb = small_pool.tile([P, B], f32, tag="cnt_sb")
                nc.scalar.dma_start(cnt_sb[:], cnt[:])
                recipf = small_pool.tile([P, B], f32, tag="recipf")
                nc.vector.tensor_scalar_max(recipf[:], cnt_sb[:], 1.0)
                nc.vector.reciprocal(recipf[:], recipf[:])
                recip = small_pool.tile([P, B], f16, tag="recip")
                nc.vector.tensor_scalar_mul(recip[:], recipf[:], 1.0)

                mul_eng = getattr(nc, mul_engine)

                def finalize(b0, b1):
                    if b1 <= b0:
                        return
                    mul_eng.tensor_mul(
                        A[:, b0 * D: b1 * D].rearrange(
                            "p (b d) -> p b d", d=D
                        ),
                        A[:, b0 * D: b1 * D].rearrange(
                            "p (b d) -> p b d", d=D
                        ),
                        recip[:, b0:b1, None].broadcast_to([P, b1 - b0, D]),
                    )
                    nc.scalar.dma_start(
                        out[:, b0 * D: b1 * D], A[:, b0 * D: b1 * D]
                    )

                # fin_cur[k]: next unfinalized block of slot k's range
                fin_cur = [M[k] for k in range(nslots)]

                # stream the JDS array; each slot-aligned segment adds into
                # A (copy for the first slot).  When a slot's stream passes
                # the end of a finalize batch, multiply+store it.
                for t in range(len(bounds) - 1):
                    blk0, blk1 = bounds[t], bounds[t + 1]
                    w = blk1 - blk0
                    tl = stream_pool.tile(
                        [P, chunk_blocks * D], f16, tag="stream"
                    )
                    nc.sync.dma_start(
                        tl[:, : w * D], ejds[:, blk0 * D: blk1 * D]
                    )
                    k = int(np.searchsorted(Cs, blk0, side="right")) - 1
                    while k < nslots and Cs_l[k] < blk1:
                        s0 = max(blk0, Cs_l[k])
                        s1 = min(blk1, Cs_l[k + 1])
                        if s1 > s0:
                            alo = (s0 - Cs_l[k]) * D
                            if k == 0 and use_copy:
                                nc.vector.tensor_copy(
                                    A[:, alo: alo + (s1 - s0) * D],
                                    tl[:, (s0 - blk0) * D: (s1 - blk0) * D],
                                )
                            else:
                                nc.vector.tensor_add(
                                    A[:, alo: alo + (s1 - s0) * D],
                                    A[:, alo: alo + (s1 - s0) * D],
                                    tl[:, (s0 - blk0) * D: (s1 - blk0) * D],
                                )
                            # progressive finalize inside this slot's range
                            reached = min(s1 - Cs_l[k], Bs_l[k])
                            if reached > fin_cur[k]:
                                done = s1 == Cs_l[k + 1]
                                if done or (
                                    reached - fin_cur[k] >= min_fin_blocks
                                ):
                                    finalize(fin_cur[k], reached)
                                    fin_cur[k] = reached
                        k += 1
                if not use_copy:
                    # blocks above every slot's coverage are pure zeros
                    top = max(Bs_l) if Bs_l else 0
                    finalize(top, B)

            if loop_repeats is not None:
                with tc.For_i(0, loop_repeats, 1):
                    emit_body()
            else:
                for _rep in range(repeats):
                    emit_body()
    _split_multi_waits(nc)
    return nc


def _make_runner(nc, in_maps):
    """Build a repeat-callable PJRT runner with inputs staged on-device once.

    Mirrors bass2jax.run_bass_via_pjrt's multi-core path, minus output-buffer
    donation (so the staged arrays can be reused across timing calls).
    """
    import jax
    from jax.experimental.shard_map import shard_map
    from jax.sharding import Mesh, NamedSharding, PartitionSpec

    from concourse import bass2jax

    bass2jax.install_neuronx_cc_hook()
    n_cores = len(in_maps)

    partition_name = (
        nc.partition_id_tensor.name if nc.partition_id_tensor else None
    )
    in_names, out_names, out_avals, zero_outs = [], [], [], []
    for alloc in nc.m.functions[0].allocations:
        if not isinstance(alloc, mybir.MemoryLocationSet):
            continue
        name = alloc.memorylocations[0].name
        if alloc.kind == "ExternalInput":
            if name != partition_name:
                in_names.append(name)
        elif alloc.kind == "ExternalOutput":
            out_names.append(name)
            shape = tuple(alloc.tensor_shape)
            dtype = mybir.dt.np(alloc.dtype)
            out_avals.append(jax.core.ShapedArray(shape, dtype))
            zero_outs.append(np.zeros(shape, dtype))
    n_params = len(in_names)
    all_names = in_names + out_names
    if partition_name is not None:
        all_names = all_names + [partition_name]

    def _body(*args):
        operands = list(args)
        if partition_name is not None:
            operands.append(bass2jax.partition_id_tensor())
        outs = bass2jax._bass_exec_p.bind(
            *operands,
            out_avals=tuple(out_avals),
            in_names=tuple(all_names),
            out_names=tuple(out_names),
            lowering_input_output_aliases=(),
            sim_require_finite=True,
            sim_require_nnan=True,
            nc=nc,
        )
        return tuple(outs)

    devices = jax.devices()[:n_cores]
    mesh = Mesh(np.asarray(devices), ("core",))
    nmaps = n_params + len(out_names)
    sharded = jax.jit(
        shard_map(
            _body,
            mesh=mesh,
            in_specs=(PartitionSpec("core"),) * nmaps,
            out_specs=(PartitionSpec("core"),) * len(out_names),
            check_rep=False,
        ),
        keep_unused=True,
    )
    sh = NamedSharding(mesh, PartitionSpec("core"))
    staged = [
        jax.device_put(
            np.concatenate([np.asarray(m[name]) for m in in_maps], axis=0), sh
        )
        for name in in_names
    ] + [
        jax.device_put(
            np.zeros((n_cores * z.shape[0], *z.shape[1:]), z.dtype), sh
        )
        for z in zero_outs
    ]

    def run(full=False):
        outs = sharded(*staged)
        if full:
            return [np.asarray(o) for o in outs]
        # under axon, block_until_ready alone doesn't track remote
        # completion reliably -- read back one shard as a completion token
        # (small, so readback noise stays out of the timing)
        return [np.asarray(o.addressable_shards[0].data) for o in outs]

    return run


def kernel(e, dst, n_nodes):
    global LAST_RESULT
    e = np.ascontiguousarray(np.asarray(e), dtype=np.float32)
    dst = np.asarray(dst).astype(np.int64)
    assert int(n_nodes) == N and e.shape == (E, D) and dst.shape == (E,)

    e_jds, cnt, order, Bs, Cs, SumB, maxdeg, B, m = _preprocess(e, dst)

    nc = _build_program(SumB, Bs, Cs, maxdeg, B)
    in_maps = [
        {"ejds": e_jds[c].reshape(P, SumB * D), "cnt": cnt[c]}
        for c in range(NCORES)
    ]
    res = run_bass_kernel_spmd(
        nc,
        in_maps,
        core_ids=list(range(NCORES)),
        trace=TRACE,
        **TRACE_KWARGS,
    )
    LAST_RESULT = res

    out_full = np.zeros((N, D), np.float32)
    ranks = np.arange(m, dtype=np.int64)
    for c in range(NCORES):
        A = np.asarray(res.results[c]["out"]).astype(np.float32)
        A = A.reshape(P, B, D)
        # rank r lives at [r % P, r // P]; rank r is node order[8r + c]
        vals = A.transpose(1, 0, 2).reshape(B * P, D)[:m]
        out_full[order[c + NCORES * ranks]] = vals
    return out_full


def benchmark(e, dst, n_nodes, r_lo=8, r_hi=308, calls=12, **build_kw):
    """Estimate steady-state per-invocation HW time via the slope method:
    two programs with the kernel body repeated r_lo / r_hi times in an
    on-device For_i loop; d(wall)/dR isolates on-device time from the
    RPC/staging constant.  The two programs' timing calls are INTERLEAVED
    so slow drift in the ~90 ms axon round-trip constant cancels instead
    of biasing the slope.  Returns (ns_per_invocation, details_dict)."""
    import time

    e = np.ascontiguousarray(np.asarray(e), dtype=np.float32)
    dst = np.asarray(dst).astype(np.int64)
    e_jds, cnt, order, Bs, Cs, SumB, maxdeg, B, m = _preprocess(e, dst)
    in_maps = [
        {"ejds": e_jds[c].reshape(P, SumB * D), "cnt": cnt[c]}
        for c in range(NCORES)
    ]

    runners = {}
    for R in (r_lo, r_hi):
        nc = _build_program(SumB, Bs, Cs, maxdeg, B, loop_repeats=R, **build_kw)
        runners[R] = _make_runner(nc, in_maps)
        runners[R]()  # compile + warmup
        runners[R]()

    results = {r_lo: [], r_hi: []}
    for _ in range(calls):
        for R in (r_lo, r_hi):
            t0 = time.perf_counter()
            runners[R]()
            results[R].append(time.perf_counter() - t0)
    for R in (r_lo, r_hi):
        print(f"R={R}: times(ms) = "
              f"{[f'{t*1e3:.2f}' for t in sorted(results[R])]}")

    # pair consecutive (lo, hi) calls and take the best-slope quantile to
    # reject stragglers on either side
    deltas = sorted(
        (hi - lo) / (r_hi - r_lo)
        for lo, hi in zip(results[r_lo], results[r_hi])
    )
    tau = deltas[len(deltas) // 4]
    tau_minmin = (min(results[r_hi]) - min(results[r_lo])) / (r_hi - r_lo)
    print(f"slope(q25 paired) = {tau*1e9:.0f} ns, "
          f"slope(min-min) = {tau_minmin*1e9:.0f} ns")
    return tau * 1e9, results
